# revision 31
# baseline (speedup 1.0000x reference)
# Multi-head self-attention (B=4, S=2048, D=1024, H=16) on 8 TRN2 NeuronCores.
#
# Sharding: batch x head-group data/tensor parallel. Core c handles batch
# b = c//2 and head group g = c%2 (8 of 16 heads, 512 of 1024 hidden dims).
# Each core computes Q/K/V projections for its local heads, attention for its
# 8 (batch, head) pairs, and a partial output projection over its 512 local
# dims. The two cores sharing a batch produce partial [D, S] outputs that the
# host sums, transposes, and biases.
#
# Per-core design (all SBUF-resident after load; bf16 matmuls, fp32 PSUM):
#   xT  [128(dp), 8(dc), 2048(s)]   x[b].T, d on partitions
#   qT/kT [128(mp), 4(mt), 2048(s)] transposed projections; partitions of
#                                   m-tile mt = heads 2mt (rows 0-63) and
#                                   2mt+1 (rows 64-127)
#   scores^T [ki, qi] per head: no max subtraction needed (scores ~ N(0,1)),
#   exp on ScalarE with fused 1/sqrt(Dh) scale, att@v without transposes.
#   The softmax denominator Z rides inside the att@v matmul: each head's
#   128-wide stationary v-block embeds a ones column (even head: v|1@64|0,
#   av rows 0-63, Z row 64; odd head: 0|1@32|v, av rows 64-127, Z row 32),
#   so av and Z accumulate in one matmul per head per ki tile.
#
#   The attention inner loop is ACT(exp)-bound: one [128,1024] exp per ki
#   tile. PE work is software-pipelined around it: av matmuls lag scores/exp
#   by 2 iterations (crossing chunk boundaries), chunk epilogues run as
#   deferred queue items during the next chunk, and the next m-tile's Q/K
#   projection matmuls are dribbled one per iteration into the PE slack.

import numpy as np
import ml_dtypes

B, S, D = 4, 2048, 1024
H, DH = 16, 64
NCORES = 8
GROUPS = 2            # head groups (cores per batch)
ML = D // GROUPS      # 512 local output dims per core
DC = D // 128         # 8 contraction chunks for projections
MT = ML // 128        # 4 m-tiles (2 heads each)
SC = S // 512         # 4 sequence chunks of 512
ST = S // 128         # 16 sequence tiles of 128
NT = D // 128         # 8 output-dim tiles
HL = H // GROUPS      # 8 local heads

BF16 = ml_dtypes.bfloat16

_nc_cache = None
LAST_RESULTS = None   # BassKernelResults of the most recent run (for test.py)


def _build_program():
    import concourse.mybir as mybir
    import concourse.tile as tile
    from concourse import bacc

    f32 = mybir.dt.float32
    bf16 = mybir.dt.bfloat16
    Exp = mybir.ActivationFunctionType.Exp

    nc = bacc.Bacc("TRN2", target_bir_lowering=False, debug=False)

    xT_d = nc.declare_dram_parameter("xT", [D, S], bf16, isOutput=False)
    wqT_d = nc.declare_dram_parameter("wqT", [D, ML], bf16, isOutput=False)
    wkT_d = nc.declare_dram_parameter("wkT", [D, ML], bf16, isOutput=False)
    wvT_d = nc.declare_dram_parameter("wvT", [D, ML], bf16, isOutput=False)
    woT_d = nc.declare_dram_parameter("woT", [ML, D], bf16, isOutput=False)
    bq_d = nc.declare_dram_parameter("bq", [1, ML], bf16, isOutput=False)
    bk_d = nc.declare_dram_parameter("bk", [1, ML], bf16, isOutput=False)
    bv_d = nc.declare_dram_parameter("bv", [1, ML], bf16, isOutput=False)
    outT_d = nc.declare_dram_parameter("outT", [D, S], bf16, isOutput=True)

    with tile.TileContext(nc) as tc:
        with (
            tc.tile_pool(name="persist", bufs=1) as persist,
            tc.tile_pool(name="work", bufs=3) as work,
            tc.tile_pool(name="ps", bufs=1, space="PSUM") as ps,
        ):
            # ---- persistent SBUF tensors ----
            xT = persist.tile([128, DC, S], bf16)
            wq = persist.tile([128, DC, ML], bf16)
            wk = persist.tile([128, DC, ML], bf16)
            wv = persist.tile([128, DC, ML], bf16)
            wo = persist.tile([128, MT, D], bf16)

            def load_xq(h):
                nc.sync.dma_start(
                    out=xT[:, 2 * h:2 * h + 2, :],
                    in_=xT_d[h * 256:(h + 1) * 256, :].rearrange("(c p) s -> p c s", p=128),
                )
            # order by first consumer; three dispatch engines for parallel queues
            nc.gpsimd.dma_start(out=wv, in_=wvT_d[:, :].rearrange("(c p) s -> p c s", p=128))
            load_xq(0)                                      # SP queue
            nc.scalar.dma_start(out=xT[:, 2:4, :],
                                in_=xT_d[256:512, :].rearrange("(c p) s -> p c s", p=128))
            nc.gpsimd.dma_start(out=xT[:, 4:6, :],
                                in_=xT_d[512:768, :].rearrange("(c p) s -> p c s", p=128))
            nc.scalar.dma_start(out=xT[:, 6:8, :],
                                in_=xT_d[768:1024, :].rearrange("(c p) s -> p c s", p=128))
            nc.sync.dma_start(out=wk, in_=wkT_d[:, :].rearrange("(c p) s -> p c s", p=128))
            nc.gpsimd.dma_start(out=wq, in_=wqT_d[:, :].rearrange("(c p) s -> p c s", p=128))
            nc.scalar.dma_start(out=wo, in_=woT_d[:, :].rearrange("(c p) s -> p c s", p=128))
            bq_sb = persist.tile([1, ML], bf16)
            bk_sb = persist.tile([1, ML], bf16)
            bv_sb = persist.tile([1, ML], bf16)
            nc.sync.dma_start(out=bq_sb, in_=bq_d[:, :])
            nc.sync.dma_start(out=bk_sb, in_=bk_d[:, :])
            nc.sync.dma_start(out=bv_sb, in_=bv_d[:, :])

            ones_row = persist.tile([1, 128], bf16)   # K=1 lhsT for bv broadcast
            nc.vector.memset(ones_row, 1.0)
            ones_rz = persist.tile([65, 64], bf16)    # rows 32/64: K=1 lhsT for 1/Z bcast
            nc.vector.memset(ones_rz, 1.0)
            ones_s = persist.tile([1, 512], bf16)     # K=1 rhs for q/k bias rows
            nc.vector.memset(ones_s, 1.0)

            qT = persist.tile([128, MT, S], bf16)
            kT = persist.tile([128, MT, S], bf16)
            # v with embedded softmax-denominator column (see header)
            v_sb = persist.tile([128, ST, HL, 128], bf16)
            avn = persist.tile([128, MT, S], bf16)    # normalized att@v, [m, s]
            bvb = persist.tile([128, ML], f32)        # bv broadcast to 128 rows

            # ---- one-time: broadcast bv across partitions via K=1 matmul ----
            p_bv = ps.tile([128, ML], f32, tag="pq", bufs=1)
            nc.tensor.matmul(out=p_bv, lhsT=ones_row, rhs=bv_sb, start=True, stop=True)
            nc.vector.tensor_copy(out=bvb, in_=p_bv)

            # ---- V projection, natural layout [s, m] ----
            # zeros/ones init on the otherwise-idle GPSIMD engine, split so
            # the first v-tiles are ready early
            for half in range(2):
                sl = slice(half * (ST // 2), (half + 1) * (ST // 2))
                nc.gpsimd.memset(v_sb[:, sl, :, :], 0.0)
                nc.gpsimd.memset(v_sb[:, sl, 0:HL:2, 64:65], 1.0)
                nc.gpsimd.memset(v_sb[:, sl, 1:HL:2, 32:33], 1.0)
            bvb3 = bvb.rearrange("p (h d) -> p h d", h=HL)

            def v_group(st):
                pv = ps.tile([128, 512], f32, tag="scores", bufs=2, name="pv")
                for dc in range(DC):
                    nc.tensor.matmul(
                        out=pv,
                        lhsT=xT[:, dc, st * 128:(st + 1) * 128],
                        rhs=wv[:, dc, :],
                        start=(dc == 0), stop=(dc == DC - 1),
                    )
                pv3 = pv.rearrange("p (h d) -> p h d", h=HL)
                nc.vector.tensor_add(
                    out=v_sb[:, st, 0:HL:2, 0:64], in0=pv3[:, 0:HL:2, :],
                    in1=bvb3[:, 0:HL:2, :],
                )
                nc.vector.tensor_add(
                    out=v_sb[:, st, 1:HL:2, 64:128], in0=pv3[:, 1:HL:2, :],
                    in1=bvb3[:, 1:HL:2, :],
                )

            # ---- Q/K projection streams: one PE instruction per step so the
            # work dribbles into attention's ACT-bound slack ----
            def make_qk_stream(mt, tasks=None):
                if tasks is None:
                    tasks = [
                        (w_sb, b_sb, dest, sc)
                        for w_sb, b_sb, dest in ((wq, bq_sb, qT), (wk, bk_sb, kT))
                        for sc in range(SC)
                    ]
                state = {"i": 0, "dc": 0, "pq": None}

                def step():
                    if state["i"] >= len(tasks):
                        return False
                    w_sb, b_sb, dest, sc = tasks[state["i"]]
                    dc = state["dc"]
                    if dc == 0:
                        state["pq"] = ps.tile([128, 512], f32, tag="pq", bufs=1,
                                              name="pq")
                    pq = state["pq"]
                    if dc < DC:
                        nc.tensor.matmul(
                            out=pq,
                            lhsT=w_sb[:, dc, mt * 128:(mt + 1) * 128],
                            rhs=xT[:, dc, sc * 512:(sc + 1) * 512],
                            start=(dc == 0), stop=False,
                        )
                        state["dc"] += 1
                    else:
                        # bias row via K=1 matmul: pq[m, s] += b[m] * 1
                        nc.tensor.matmul(
                            out=pq,
                            lhsT=b_sb[:, mt * 128:(mt + 1) * 128],
                            rhs=ones_s,
                            start=False, stop=True,
                        )
                        nc.vector.tensor_copy(
                            out=dest[:, mt, sc * 512:(sc + 1) * 512], in_=pq,
                        )
                        state["dc"] = 0
                        state["i"] += 1
                    return True

                def flush():
                    while step():
                        pass
                return step, flush

            # ---- attention with a global deferred-work queue: av matmuls lag
            # scores/exp by 2 iterations across chunk boundaries ----
            avq = []

            def drain_avq(target):
                while len(avq) > target:
                    avq.pop(0)()

            def attention_chunk(mt, qi, inject=None, stream=None):
                pav0 = ps.tile([128, 512], f32, tag="av", bufs=3, name="pav0")
                pav1 = ps.tile([128, 512], f32, tag="av", bufs=3, name="pav1")
                eTs = {}

                def av_z(kt):
                    eT = eTs.pop(kt)
                    # att@v + Z in one matmul per head (ones column baked into
                    # the 128-wide stationary; zero cols give zero rows)
                    nc.tensor.matmul(
                        out=pav0,
                        lhsT=v_sb[:, kt, 2 * mt, :],
                        rhs=eT[:, 0:512],
                        start=(kt == 0), stop=(kt == ST - 1),
                    )
                    nc.tensor.matmul(
                        out=pav1,
                        lhsT=v_sb[:, kt, 2 * mt + 1, :],
                        rhs=eT[:, 512:1024],
                        start=(kt == 0), stop=(kt == ST - 1),
                    )

                def epilogue():
                    # normalize: avn = av * (1/Z); av copied to SBUF right
                    # away (frees the PSUM accumulators), 1/Z broadcast via
                    # K=1 matmuls, muls read broadcast from PSUM + av from SBUF
                    av_s = work.tile([128, 1024], f32, tag="av_s", bufs=3)
                    nc.vector.tensor_copy(out=av_s[:, 0:512], in_=pav0)
                    nc.vector.tensor_copy(out=av_s[:, 512:1024], in_=pav1)
                    rz = work.tile([65, 1024], bf16, tag="rz", bufs=3)
                    with nc.allow_low_precision(reason="1/Z in bf16 matches bf16 att"):
                        nc.vector.reciprocal(out=rz[64:65, 0:512],
                                             in_=av_s[64:65, 0:512])
                        nc.vector.reciprocal(out=rz[32:33, 512:1024],
                                             in_=av_s[32:33, 512:1024])
                    przb = ps.tile([128, 512], f32, tag="pq", bufs=1, name="przb")
                    nc.tensor.matmul(
                        out=przb[0:64, :], lhsT=ones_rz[64:65, :],
                        rhs=rz[64:65, 0:512], start=True, stop=True,
                    )
                    nc.tensor.matmul(
                        out=przb[64:128, :], lhsT=ones_rz[32:33, :],
                        rhs=rz[32:33, 512:1024], start=True, stop=True,
                    )
                    nc.vector.tensor_mul(
                        out=avn[0:64, mt, qi * 512:(qi + 1) * 512],
                        in0=av_s[0:64, 0:512], in1=przb[0:64, :],
                    )
                    nc.vector.tensor_mul(
                        out=avn[64:128, mt, qi * 512:(qi + 1) * 512],
                        in0=av_s[64:128, 512:1024], in1=przb[64:128, :],
                    )

                for kt in range(ST):
                    pscore = ps.tile([128, 1024], f32, tag="scores", bufs=2)
                    # scores^T [ki, qi] for both heads, row-packed (K=64)
                    nc.tensor.matmul(
                        out=pscore[:, 0:512],
                        lhsT=kT[0:64, mt, kt * 128:(kt + 1) * 128],
                        rhs=qT[0:64, mt, qi * 512:(qi + 1) * 512],
                        start=True, stop=True,
                    )
                    nc.tensor.matmul(
                        out=pscore[:, 512:1024],
                        lhsT=kT[64:128, mt, kt * 128:(kt + 1) * 128],
                        rhs=qT[64:128, mt, qi * 512:(qi + 1) * 512],
                        start=True, stop=True,
                    )
                    eT = work.tile([128, 1024], bf16, tag="eT", bufs=6)
                    nc.scalar.activation(out=eT, in_=pscore, func=Exp,
                                         scale=DH ** -0.5)
                    eTs[kt] = eT
                    if inject and kt in inject:
                        for thunk in inject[kt]:
                            thunk()
                    if stream is not None:
                        stream()
                    avq.append(lambda k=kt: av_z(k))
                    drain_avq(2)
                avq.append(epilogue)

            # ---- orchestrate ----
            for st in range(4):
                v_group(st)
            _, qk0a_flush = make_qk_stream(0, tasks=(
                [(wk, bk_sb, kT, sc) for sc in range(SC)] + [(wq, bq_sb, qT, 0)]))
            qk0a_flush()
            _, qk0b_flush = {}, {}
            qk0b = [make_qk_stream(0, tasks=[(wq, bq_sb, qT, sc)])[1]
                    for sc in range(1, SC)]
            # chunk (0,0) injections: remaining v groups one per kt, and the
            # q(sc=1..3) groups late in the chunk (needed from chunk (0,1) on)
            inject0 = {kt: [(lambda s=kt + 4: v_group(s))] for kt in range(12)}
            inject0.setdefault(9, []).append(qk0b[0])
            inject0.setdefault(11, []).append(qk0b[1])
            inject0.setdefault(13, []).append(qk0b[2])
            streams = {mt: make_qk_stream(mt + 1) for mt in range(MT - 1)}
            for mt in range(MT):
                step = streams[mt][0] if mt in streams else None
                for qi in range(SC):
                    attention_chunk(mt, qi, inject=inject0 if (mt, qi) == (0, 0) else None,
                                    stream=step)
                if mt in streams:
                    streams[mt][1]()   # flush remaining proj steps
            drain_avq(0)

            # ---- partial output projection: outT[n, s] = woT.T @ avn ----
            for nt in range(NT):
                for sc in range(SC):
                    po = ps.tile([128, 512], f32, tag="scores", bufs=2)
                    for mt in range(MT):
                        nc.tensor.matmul(
                            out=po,
                            lhsT=wo[:, mt, nt * 128:(nt + 1) * 128],
                            rhs=avn[:, mt, sc * 512:(sc + 1) * 512],
                            start=(mt == 0), stop=(mt == MT - 1),
                        )
                    osb = work.tile([128, 512], bf16, tag="osb", bufs=3)
                    nc.vector.tensor_copy(out=osb, in_=po)
                    nc.sync.dma_start(
                        out=outT_d[nt * 128:(nt + 1) * 128, sc * 512:(sc + 1) * 512],
                        in_=osb,
                    )

    nc.compile()
    return nc


def _get_nc():
    global _nc_cache
    if _nc_cache is None:
        _nc_cache = _build_program()
    return _nc_cache


def _core_inputs(x, Wq, bq, Wk, bk, Wv, bv, Wo, c):
    b, g = divmod(c, GROUPS)
    sl = slice(g * ML, (g + 1) * ML)
    return {
        "xT": np.ascontiguousarray(x[b].T).astype(BF16),
        "wqT": np.ascontiguousarray(Wq[sl].T).astype(BF16),
        "wkT": np.ascontiguousarray(Wk[sl].T).astype(BF16),
        "wvT": np.ascontiguousarray(Wv[sl].T).astype(BF16),
        "woT": np.ascontiguousarray(Wo[:, sl].T).astype(BF16),
        "bq": bq[sl].reshape(1, ML).astype(BF16),
        "bk": bk[sl].reshape(1, ML).astype(BF16),
        "bv": bv[sl].reshape(1, ML).astype(BF16),
    }


def kernel(x, Wq, bq, Wk, bk, Wv, bv, Wo, bo, **kwargs):
    global LAST_RESULTS
    from concourse.bass_utils import run_bass_kernel_spmd

    x = np.asarray(x, dtype=np.float32)
    nc = _get_nc()
    in_maps = [
        _core_inputs(x, np.asarray(Wq), np.asarray(bq), np.asarray(Wk),
                     np.asarray(bk), np.asarray(Wv), np.asarray(bv),
                     np.asarray(Wo), c)
        for c in range(NCORES)
    ]
    res = run_bass_kernel_spmd(nc, in_maps, core_ids=list(range(NCORES)), **kwargs)
    LAST_RESULTS = res

    bo = np.asarray(bo, dtype=np.float32)
    out = np.empty((B, S, D), dtype=np.float32)
    for b in range(B):
        partial = (res.results[2 * b]["outT"].astype(np.float32)
                   + res.results[2 * b + 1]["outT"].astype(np.float32))
        out[b] = partial.T + bo
    return out


# revision 41
# speedup vs baseline: 1.0190x; 1.0190x over previous
# Multi-head self-attention (B=4, S=2048, D=1024, H=16) on 8 TRN2 NeuronCores.
#
# Sharding: batch x head-group data/tensor parallel. Core c handles batch
# b = c//2 and head group g = c%2 (8 of 16 heads, 512 of 1024 hidden dims).
# Each core computes Q/K/V projections for its local heads, attention for its
# 8 (batch, head) pairs, and a partial output projection over its 512 local
# dims. The two cores sharing a batch produce partial [D, S] outputs that the
# host sums, transposes, and biases.
#
# Per-core design (all SBUF-resident after load; bf16 matmuls, fp32 PSUM):
#   xT  [128(dp), 8(dc), 2048(s)]   x[b].T, d on partitions
#   qT/kT [128(mp), 4(mt), 2048(s)] transposed projections; partitions of
#                                   m-tile mt = heads 2mt (rows 0-63) and
#                                   2mt+1 (rows 64-127)
#   scores^T [ki, qi] per head: no max subtraction needed (scores ~ N(0,1)),
#   exp on ScalarE with fused 1/sqrt(Dh) scale, att@v without transposes.
#   The softmax denominator Z rides inside the att@v matmul: each head's
#   128-wide stationary v-block embeds a ones column (even head: v|1@64|0,
#   av rows 0-63, Z row 64; odd head: 0|1@32|v, av rows 64-127, Z row 32),
#   so av and Z accumulate in one matmul per head per ki tile.
#
#   The attention inner loop is ACT(exp)-bound: one [128,1024] exp per ki
#   tile. PE work is software-pipelined around it: av matmuls lag scores/exp
#   by 2 iterations (crossing chunk boundaries), chunk epilogues run as
#   deferred queue items during the next chunk, and the next m-tile's Q/K
#   projection matmuls are dribbled one per iteration into the PE slack.

import numpy as np
import ml_dtypes

B, S, D = 4, 2048, 1024
H, DH = 16, 64
NCORES = 8
GROUPS = 2            # head groups (cores per batch)
ML = D // GROUPS      # 512 local output dims per core
DC = D // 128         # 8 contraction chunks for projections
MT = ML // 128        # 4 m-tiles (2 heads each)
SC = S // 512         # 4 sequence chunks of 512
ST = S // 128         # 16 sequence tiles of 128
NT = D // 128         # 8 output-dim tiles
HL = H // GROUPS      # 8 local heads

BF16 = ml_dtypes.bfloat16

_nc_cache = None
LAST_RESULTS = None   # BassKernelResults of the most recent run (for test.py)


def _build_program():
    import concourse.mybir as mybir
    import concourse.tile as tile
    from concourse import bacc

    f32 = mybir.dt.float32
    bf16 = mybir.dt.bfloat16
    Exp = mybir.ActivationFunctionType.Exp

    nc = bacc.Bacc("TRN2", target_bir_lowering=False, debug=False)

    xT_d = nc.declare_dram_parameter("xT", [D, S], bf16, isOutput=False)
    wqT_d = nc.declare_dram_parameter("wqT", [D, ML], bf16, isOutput=False)
    wkT_d = nc.declare_dram_parameter("wkT", [D, ML], bf16, isOutput=False)
    wvT_d = nc.declare_dram_parameter("wvT", [D, ML], bf16, isOutput=False)
    woT_d = nc.declare_dram_parameter("woT", [ML, D], bf16, isOutput=False)
    bq_d = nc.declare_dram_parameter("bq", [1, ML], bf16, isOutput=False)
    bk_d = nc.declare_dram_parameter("bk", [1, ML], bf16, isOutput=False)
    bv_d = nc.declare_dram_parameter("bv", [1, ML], bf16, isOutput=False)
    outT_d = nc.declare_dram_parameter("outT", [D, S], bf16, isOutput=True)

    with tile.TileContext(nc) as tc:
        with (
            tc.tile_pool(name="persist", bufs=1) as persist,
            tc.tile_pool(name="work", bufs=3) as work,
            tc.tile_pool(name="ps", bufs=1, space="PSUM") as ps,
        ):
            # ---- persistent SBUF tensors ----
            xT = persist.tile([128, DC, S], bf16)
            wq = persist.tile([128, DC, ML], bf16)
            wk = persist.tile([128, DC, ML], bf16)
            wv = persist.tile([128, DC, ML], bf16)
            wo = persist.tile([128, MT, D], bf16)

            def load_xq(h):
                nc.sync.dma_start(
                    out=xT[:, 2 * h:2 * h + 2, :],
                    in_=xT_d[h * 256:(h + 1) * 256, :].rearrange("(c p) s -> p c s", p=128),
                )
            # order by first consumer; three dispatch engines for parallel queues
            nc.gpsimd.dma_start(out=wv, in_=wvT_d[:, :].rearrange("(c p) s -> p c s", p=128))
            load_xq(0)                                      # SP queue
            nc.scalar.dma_start(out=xT[:, 2:4, :],
                                in_=xT_d[256:512, :].rearrange("(c p) s -> p c s", p=128))
            nc.gpsimd.dma_start(out=xT[:, 4:6, :],
                                in_=xT_d[512:768, :].rearrange("(c p) s -> p c s", p=128))
            nc.scalar.dma_start(out=xT[:, 6:8, :],
                                in_=xT_d[768:1024, :].rearrange("(c p) s -> p c s", p=128))
            nc.sync.dma_start(out=wk, in_=wkT_d[:, :].rearrange("(c p) s -> p c s", p=128))
            nc.gpsimd.dma_start(out=wq, in_=wqT_d[:, :].rearrange("(c p) s -> p c s", p=128))
            nc.scalar.dma_start(out=wo, in_=woT_d[:, :].rearrange("(c p) s -> p c s", p=128))
            bq_sb = persist.tile([1, ML], bf16)
            bk_sb = persist.tile([1, ML], bf16)
            bv_sb = persist.tile([1, ML], bf16)
            nc.sync.dma_start(out=bq_sb, in_=bq_d[:, :])
            nc.sync.dma_start(out=bk_sb, in_=bk_d[:, :])
            nc.sync.dma_start(out=bv_sb, in_=bv_d[:, :])

            ones_row = persist.tile([1, 128], bf16)   # K=1 lhsT for bv broadcast
            nc.vector.memset(ones_row, 1.0)
            ones_rz = persist.tile([65, 64], bf16)    # rows 32/64: K=1 lhsT for 1/Z bcast
            nc.vector.memset(ones_rz, 1.0)
            ones_s = persist.tile([1, 512], bf16)     # K=1 rhs for q/k bias rows
            nc.vector.memset(ones_s, 1.0)

            qT = persist.tile([128, MT, S], bf16)
            kT = persist.tile([128, MT, S], bf16)
            # v with embedded softmax-denominator column (see header)
            v_sb = persist.tile([128, ST, HL, 128], bf16)
            avn = persist.tile([128, MT, S], bf16)    # normalized att@v, [m, s]
            bvb = persist.tile([128, ML], f32)        # bv broadcast to 128 rows

            # ---- one-time: broadcast bv across partitions via K=1 matmul ----
            p_bv = ps.tile([128, ML], f32, tag="pq", bufs=1)
            nc.tensor.matmul(out=p_bv, lhsT=ones_row, rhs=bv_sb, start=True, stop=True)
            nc.vector.tensor_copy(out=bvb, in_=p_bv)

            # ---- V projection, natural layout [s, m] ----
            # zeros/ones init on the otherwise-idle GPSIMD engine, split so
            # the first v-tiles are ready early
            for half in range(2):
                sl = slice(half * (ST // 2), (half + 1) * (ST // 2))
                nc.gpsimd.memset(v_sb[:, sl, :, :], 0.0)
                nc.gpsimd.memset(v_sb[:, sl, 0:HL:2, 64:65], 1.0)
                nc.gpsimd.memset(v_sb[:, sl, 1:HL:2, 32:33], 1.0)
            bvb3 = bvb.rearrange("p (h d) -> p h d", h=HL)

            def v_group(st):
                pv = ps.tile([128, 512], f32, tag="scores", bufs=2, name="pv")
                for dc in range(DC):
                    nc.tensor.matmul(
                        out=pv,
                        lhsT=xT[:, dc, st * 128:(st + 1) * 128],
                        rhs=wv[:, dc, :],
                        start=(dc == 0), stop=(dc == DC - 1),
                    )
                pv3 = pv.rearrange("p (h d) -> p h d", h=HL)
                nc.vector.tensor_add(
                    out=v_sb[:, st, 0:HL:2, 0:64], in0=pv3[:, 0:HL:2, :],
                    in1=bvb3[:, 0:HL:2, :],
                )
                nc.vector.tensor_add(
                    out=v_sb[:, st, 1:HL:2, 64:128], in0=pv3[:, 1:HL:2, :],
                    in1=bvb3[:, 1:HL:2, :],
                )

            # ---- Q/K projection streams: one PE instruction per step so the
            # work dribbles into attention's ACT-bound slack ----
            def make_qk_stream(mt, tasks=None):
                if tasks is None:
                    tasks = [
                        (w_sb, b_sb, dest, sc)
                        for w_sb, b_sb, dest in ((wq, bq_sb, qT), (wk, bk_sb, kT))
                        for sc in range(SC)
                    ]
                state = {"i": 0, "dc": 0, "pq": None}

                def step():
                    if state["i"] >= len(tasks):
                        return False
                    w_sb, b_sb, dest, sc = tasks[state["i"]]
                    dc = state["dc"]
                    if dc == 0:
                        state["pq"] = ps.tile([128, 512], f32, tag="pq", bufs=1,
                                              name="pq")
                    pq = state["pq"]
                    if dc < DC:
                        nc.tensor.matmul(
                            out=pq,
                            lhsT=w_sb[:, dc, mt * 128:(mt + 1) * 128],
                            rhs=xT[:, dc, sc * 512:(sc + 1) * 512],
                            start=(dc == 0), stop=False,
                        )
                        state["dc"] += 1
                    else:
                        # bias row via K=1 matmul: pq[m, s] += b[m] * 1
                        nc.tensor.matmul(
                            out=pq,
                            lhsT=b_sb[:, mt * 128:(mt + 1) * 128],
                            rhs=ones_s,
                            start=False, stop=True,
                        )
                        nc.vector.tensor_copy(
                            out=dest[:, mt, sc * 512:(sc + 1) * 512], in_=pq,
                        )
                        state["dc"] = 0
                        state["i"] += 1
                    return True

                def flush():
                    while step():
                        pass
                return step, flush

            # ---- attention with a global deferred-work queue: av matmuls lag
            # scores/exp by 2 iterations across chunk boundaries ----
            avq = []

            def drain_avq(target):
                while len(avq) > target:
                    avq.pop(0)()

            def attention_chunk(mt, qi, inject=None, stream=None):
                pav0 = ps.tile([128, 512], f32, tag="av", bufs=3, name="pav0")
                pav1 = ps.tile([128, 512], f32, tag="av", bufs=3, name="pav1")
                eTs = {}

                def av_z(kt):
                    eT = eTs.pop(kt)
                    # att@v + Z in one matmul per head (ones column baked into
                    # the 128-wide stationary; zero cols give zero rows)
                    nc.tensor.matmul(
                        out=pav0,
                        lhsT=v_sb[:, kt, 2 * mt, :],
                        rhs=eT[:, 0:512],
                        start=(kt == 0), stop=(kt == ST - 1),
                    )
                    nc.tensor.matmul(
                        out=pav1,
                        lhsT=v_sb[:, kt, 2 * mt + 1, :],
                        rhs=eT[:, 512:1024],
                        start=(kt == 0), stop=(kt == ST - 1),
                    )

                def epilogue():
                    # normalize: avn = av * (1/Z); av copied to SBUF right
                    # away (frees the PSUM accumulators), 1/Z broadcast via
                    # K=1 matmuls, muls read broadcast from PSUM + av from SBUF
                    av_s = work.tile([128, 1024], f32, tag="av_s", bufs=3)
                    nc.vector.tensor_copy(out=av_s[:, 0:512], in_=pav0)
                    nc.vector.tensor_copy(out=av_s[:, 512:1024], in_=pav1)
                    rz = work.tile([65, 1024], bf16, tag="rz", bufs=3)
                    with nc.allow_low_precision(reason="1/Z in bf16 matches bf16 att"):
                        nc.vector.reciprocal(out=rz[64:65, 0:512],
                                             in_=av_s[64:65, 0:512])
                        nc.vector.reciprocal(out=rz[32:33, 512:1024],
                                             in_=av_s[32:33, 512:1024])
                    przb = ps.tile([128, 512], f32, tag="pq", bufs=1, name="przb")
                    nc.tensor.matmul(
                        out=przb[0:64, :], lhsT=ones_rz[64:65, :],
                        rhs=rz[64:65, 0:512], start=True, stop=True,
                    )
                    nc.tensor.matmul(
                        out=przb[64:128, :], lhsT=ones_rz[32:33, :],
                        rhs=rz[32:33, 512:1024], start=True, stop=True,
                    )
                    nc.vector.tensor_mul(
                        out=avn[0:64, mt, qi * 512:(qi + 1) * 512],
                        in0=av_s[0:64, 0:512], in1=przb[0:64, :],
                    )
                    nc.vector.tensor_mul(
                        out=avn[64:128, mt, qi * 512:(qi + 1) * 512],
                        in0=av_s[64:128, 512:1024], in1=przb[64:128, :],
                    )

                for kt in range(ST):
                    pscore = ps.tile([128, 1024], f32, tag="scores", bufs=2)
                    # scores^T [ki, qi] for both heads, row-packed (K=64)
                    nc.tensor.matmul(
                        out=pscore[:, 0:512],
                        lhsT=kT[0:64, mt, kt * 128:(kt + 1) * 128],
                        rhs=qT[0:64, mt, qi * 512:(qi + 1) * 512],
                        start=True, stop=True,
                    )
                    nc.tensor.matmul(
                        out=pscore[:, 512:1024],
                        lhsT=kT[64:128, mt, kt * 128:(kt + 1) * 128],
                        rhs=qT[64:128, mt, qi * 512:(qi + 1) * 512],
                        start=True, stop=True,
                    )
                    eT = work.tile([128, 1024], bf16, tag="eT", bufs=18)
                    nc.scalar.activation(out=eT, in_=pscore, func=Exp,
                                         scale=DH ** -0.5)
                    eTs[kt] = eT
                    if inject and kt in inject:
                        for thunk in inject[kt]:
                            thunk()
                    if stream is not None:
                        stream()
                    avq.append(lambda k=kt: av_z(k))
                    drain_avq(16)
                avq.append(epilogue)

            # ---- orchestrate ----
            for st in range(4):
                v_group(st)
            _, qk0a_flush = make_qk_stream(0, tasks=(
                [(wk, bk_sb, kT, sc) for sc in range(SC)] + [(wq, bq_sb, qT, 0)]))
            qk0a_flush()
            _, qk0b_flush = {}, {}
            qk0b = [make_qk_stream(0, tasks=[(wq, bq_sb, qT, sc)])[1]
                    for sc in range(1, SC)]
            # chunk (0,0) injections: remaining v groups one per kt, and the
            # q(sc=1..3) groups late in the chunk (needed from chunk (0,1) on)
            inject0 = {kt: [(lambda s=kt + 4: v_group(s))] for kt in range(12)}
            inject0.setdefault(9, []).append(qk0b[0])
            inject0.setdefault(11, []).append(qk0b[1])
            inject0.setdefault(13, []).append(qk0b[2])
            streams = {mt: make_qk_stream(mt + 1) for mt in range(MT - 1)}
            for mt in range(MT):
                step = streams[mt][0] if mt in streams else None
                for qi in range(SC):
                    attention_chunk(mt, qi, inject=inject0 if (mt, qi) == (0, 0) else None,
                                    stream=step)
                if mt in streams:
                    streams[mt][1]()   # flush remaining proj steps
            drain_avq(0)

            # ---- partial output projection: outT[n, s] = woT.T @ avn ----
            for nt in range(NT):
                for sc in range(SC):
                    po = ps.tile([128, 512], f32, tag="scores", bufs=2)
                    for mt in range(MT):
                        nc.tensor.matmul(
                            out=po,
                            lhsT=wo[:, mt, nt * 128:(nt + 1) * 128],
                            rhs=avn[:, mt, sc * 512:(sc + 1) * 512],
                            start=(mt == 0), stop=(mt == MT - 1),
                        )
                    osb = work.tile([128, 512], bf16, tag="osb", bufs=3)
                    nc.vector.tensor_copy(out=osb, in_=po)
                    nc.sync.dma_start(
                        out=outT_d[nt * 128:(nt + 1) * 128, sc * 512:(sc + 1) * 512],
                        in_=osb,
                    )

    nc.compile()
    return nc


def _get_nc():
    global _nc_cache
    if _nc_cache is None:
        _nc_cache = _build_program()
    return _nc_cache


def _core_inputs(x, Wq, bq, Wk, bk, Wv, bv, Wo, c):
    b, g = divmod(c, GROUPS)
    sl = slice(g * ML, (g + 1) * ML)
    return {
        "xT": np.ascontiguousarray(x[b].T).astype(BF16),
        "wqT": np.ascontiguousarray(Wq[sl].T).astype(BF16),
        "wkT": np.ascontiguousarray(Wk[sl].T).astype(BF16),
        "wvT": np.ascontiguousarray(Wv[sl].T).astype(BF16),
        "woT": np.ascontiguousarray(Wo[:, sl].T).astype(BF16),
        "bq": bq[sl].reshape(1, ML).astype(BF16),
        "bk": bk[sl].reshape(1, ML).astype(BF16),
        "bv": bv[sl].reshape(1, ML).astype(BF16),
    }


def kernel(x, Wq, bq, Wk, bk, Wv, bv, Wo, bo, **kwargs):
    global LAST_RESULTS
    from concourse.bass_utils import run_bass_kernel_spmd

    x = np.asarray(x, dtype=np.float32)
    nc = _get_nc()
    in_maps = [
        _core_inputs(x, np.asarray(Wq), np.asarray(bq), np.asarray(Wk),
                     np.asarray(bk), np.asarray(Wv), np.asarray(bv),
                     np.asarray(Wo), c)
        for c in range(NCORES)
    ]
    res = run_bass_kernel_spmd(nc, in_maps, core_ids=list(range(NCORES)), **kwargs)
    LAST_RESULTS = res

    bo = np.asarray(bo, dtype=np.float32)
    out = np.empty((B, S, D), dtype=np.float32)
    for b in range(B):
        partial = (res.results[2 * b]["outT"].astype(np.float32)
                   + res.results[2 * b + 1]["outT"].astype(np.float32))
        out[b] = partial.T + bo
    return out
